# revision 18
# baseline (speedup 1.0000x reference)
"""BiasAttention TRN2 kernel — q-sharded across 8 NeuronCores, fp8 z.

Each core owns a block of 128 queries and computes the full attention for
them (all 8 heads, all 1024 keys).  The z-bias projection dominates both
HBM traffic and PE time, so z ships as fp8e3 (e3m4: 4 mantissa bits keeps
the bias quantization error ~5x below the bf16-path output error budget)
— halving DMA bytes vs bf16 and doubling LDWEIGHTS rate via fast weight
load.  Attention runs in transposed form S^T[k, q] so that P^T = exp(S^T)
is directly the stationary operand for attn.V — no PE transposes of P.
The z.Wb matmuls and the K^T.Q matmuls accumulate into one PSUM tile
[k, q, h]; ACT applies exp straight out of PSUM.  The bias-projection
bias bb and the K half of bkv are constant along the softmax axis and
cancel — they are dropped.
"""

import sys

if "/opt/trn_rl_repo" not in sys.path:
    sys.path.insert(0, "/opt/trn_rl_repo")

import ml_dtypes
import numpy as np

import concourse.bass as bass
import concourse.mybir as mybir
from concourse import bacc
from concourse.bass_utils import run_bass_kernel_spmd
from concourse.masks import make_identity
from concourse.tile import TileContext

P = 128          # partitions
H = 8            # heads
D = 32           # head dim
CQ = 256         # q channels
CKV = 256        # kv channels
BD = 128         # bias (z) channels
NQ = 1024        # total queries
NK = 1024        # total keys
NCORES = 8
NQC = NQ // NCORES   # queries per core = 128
KC_N = NK // P       # k-chunks = 8
G = 2                # z DMA groups per k-chunk
TQ = NQC // G        # q-tiles per group = 64
SCALE = D ** (-0.5)

FP = mybir.dt.float32
BF = mybir.dt.bfloat16
F8 = mybir.dt.float8e3
NP_BF = ml_dtypes.bfloat16
NP_F8 = ml_dtypes.float8_e3m4


def build_program():
    add = mybir.AluOpType.add
    mult = mybir.AluOpType.mult

    nc = bacc.Bacc("TRN2", target_bir_lowering=False, debug=False,
                   num_devices=NCORES)

    # ---- I/O ----
    # zT[kc, g, c, tq, k] = z[q = g*TQ + tq, kc*P + k, c]
    zT = nc.dram_tensor("zT", [KC_N, G, BD, TQ, P], F8, kind="ExternalInput")
    xqT = nc.dram_tensor("xqT", [CQ, NQC], BF, kind="ExternalInput")
    xkvT = nc.dram_tensor("xkvT", [CKV, NK], BF, kind="ExternalInput")
    Wq = nc.dram_tensor("Wq", [CQ, H * D], BF, kind="ExternalInput")
    bq = nc.dram_tensor("bq", [H * D], FP, kind="ExternalInput")
    Wkv = nc.dram_tensor("Wkv", [CKV, 2 * H * D], BF, kind="ExternalInput")
    bkvV = nc.dram_tensor("bkvV", [H * D], BF, kind="ExternalInput")
    Wb = nc.dram_tensor("Wb", [BD, H], BF, kind="ExternalInput")
    WpBF = nc.dram_tensor("WpBF", [H * D, CQ], BF, kind="ExternalInput")
    bpBF = nc.dram_tensor("bpBF", [CQ], BF, kind="ExternalInput")
    y = nc.dram_tensor("y", [NQC, CQ], FP, kind="ExternalOutput")

    with TileContext(nc) as tc:
        with (
            tc.tile_pool(name="const", bufs=1) as const,
            tc.tile_pool(name="zpool", bufs=KC_N * G) as zpool,
            tc.tile_pool(name="xpool", bufs=4) as xpool,
            tc.tile_pool(name="proj_ps", bufs=3, space="PSUM") as proj_ps,
            tc.tile_pool(name="b_ps", bufs=4, space="PSUM") as b_psp,
            tc.tile_pool(name="o_ps", bufs=1, space="PSUM") as o_psp,
        ):
            # ---- constants / weights to SBUF ----
            wb_sb = const.tile([P, H], BF)
            nc.sync.dma_start(wb_sb, Wb[:])
            wq_sb = const.tile([P, 2, H * D], BF)
            nc.sync.dma_start(wq_sb, Wq.rearrange("(o p) m -> p o m", p=P))
            wkv_sb = const.tile([P, 2, 2 * H * D], BF)
            nc.sync.dma_start(wkv_sb, Wkv.rearrange("(o p) m -> p o m", p=P))
            wp_sb = const.tile([P, 2, CQ], BF)
            nc.sync.dma_start(wp_sb, WpBF.rearrange("(o p) m -> p o m", p=P))
            xqT_sb = const.tile([P, 2, NQC], BF)
            nc.sync.dma_start(xqT_sb, xqT.rearrange("(o p) q -> p o q", p=P))
            xkvT_sb = const.tile([P, 2, NK], BF)
            nc.sync.dma_start(xkvT_sb, xkvT.rearrange("(o p) k -> p o k", p=P))
            bq_sb = const.tile([P, 2], FP)
            nc.sync.dma_start(bq_sb, bq.rearrange("(o p) -> p o", p=P))
            bkvV_sb = const.tile([1, H * D], BF)
            nc.sync.dma_start(bkvV_sb, bkvV[None, :])
            bp_sb = const.tile([1, CQ], BF)
            nc.sync.dma_start(bp_sb, bpBF[None, :])
            # all 16 z groups resident: every DMA issued up front so the
            # DMA engines stream back-to-back at line rate.  The first two
            # groups are split into 256KB quarters so the bias matmuls can
            # begin as soon as the first piece lands.
            # alternate between two DMA queues (Sync HWDGE / GpSimd SWDGE)
            # so the z stream isn't capped by a single ring.  Not the
            # Scalar ring: dma_starts there would block the ACT FIFO and
            # stall the exps behind them.
            zpre = []
            for gidx in range(KC_N * G):
                eng = nc.sync if gidx % 2 == 0 else nc.gpsimd
                z_sb = zpool.tile([P, TQ, P], F8, tag="z", name=f"z{gidx}")
                if gidx < 2:
                    sq = TQ // 4
                    for s in range(4):
                        eng.dma_start(
                            z_sb[:, s * sq:(s + 1) * sq, :],
                            zT[gidx // G, gidx % G, :, s * sq:(s + 1) * sq, :])
                else:
                    eng.dma_start(z_sb, zT[gidx // G, gidx % G])
                zpre.append(z_sb)
            ident_bf = const.tile([P, P], BF)
            make_identity(nc, ident_bf)
            ones_row = const.tile([1, P], BF)
            nc.vector.memset(ones_row, 1.0)

            # V augmented with a ones column per head: [k, kc, h, D+1]
            vaug_sb = const.tile([P, KC_N, H, D + 1], BF)
            nc.vector.memset(vaug_sb, 1.0)

            # ---- projections (bf16 in, fp32 psum accumulate) ----
            # Q block-diag: qblk[(hl d), m, q*H + h] = Q_scaled[q, h, d] when
            # h == m*4 + hl, else 0.  Makes S^T for all 8 heads two plain
            # N=512 matmuls per (k-chunk, q-half) with output [k, (q, h)]
            # contiguous — the same layout the bias matmuls accumulate into.
            qblk_sb = const.tile([P, 2, NQC * H], BF)
            nc.vector.memset(qblk_sb, 0.0)
            qblk_v = qblk_sb.rearrange("p m (q h) -> p m q h", h=H)
            for m in range(2):
                ps = proj_ps.tile([P, 512], FP, tag="proj")
                for c in range(2):
                    nc.tensor.matmul(ps[:, :NQC],
                                     lhsT=wq_sb[:, c, m * P:(m + 1) * P],
                                     rhs=xqT_sb[:, c, :],
                                     start=(c == 0), stop=(c == 1))
                for hl in range(4):
                    r0 = hl * 32
                    nc.vector.tensor_scalar(
                        qblk_v[r0:r0 + 32, m, :, m * 4 + hl],
                        ps[r0:r0 + 32, :NQC],
                        bq_sb[r0:r0 + 32, m:m + 1], SCALE, add, mult)

            # K^T [(h d), k], no bias (cancels in softmax), stored bf16
            kT_sb = const.tile([P, 2, NK], BF)
            for m in range(2):
                for nh in range(NK // 512):
                    ps = proj_ps.tile([P, 512], FP, tag="proj")
                    for c in range(2):
                        nc.tensor.matmul(ps[:, :],
                                         lhsT=wkv_sb[:, c, m * P:(m + 1) * P],
                                         rhs=xkvT_sb[:, c, nh * 512:(nh + 1) * 512],
                                         start=(c == 0), stop=(c == 1))
                    nc.vector.tensor_copy(
                        kT_sb[:, m, nh * 512:(nh + 1) * 512], ps)

            # V [k, (h d)] + bkv_V, written into vaug (ones col preserved)
            for kc in range(KC_N):
                ps = proj_ps.tile([P, 512], FP, tag="proj", name="v_ps")
                for c in range(2):
                    nc.tensor.matmul(ps[:, :H * D],
                                     lhsT=xkvT_sb[:, c, kc * P:(kc + 1) * P],
                                     rhs=wkv_sb[:, c, H * D:2 * H * D],
                                     start=(c == 0), stop=False)
                nc.tensor.matmul(ps[:, :H * D], lhsT=ones_row,
                                 rhs=bkvV_sb, start=False, stop=True)
                nc.vector.tensor_copy(
                    vaug_sb[:, kc, :, 0:D],
                    ps[:, :H * D].rearrange("p (h d) -> p h d", h=H))

            # ---- main loop over k-chunks: S^T + bias accumulate in PSUM ----
            o_ps = o_psp.tile([P, H * (D + 1)], FP)   # [q, h*(D+1)]
            av_queue = []

            def av_emit(item, last):
                xp, kp = item
                xv = xp.rearrange("p (q h) -> p h q", h=H)
                for h in range(H):
                    nc.tensor.matmul(
                        o_ps[:, h * (D + 1):(h + 1) * (D + 1)],
                        lhsT=xv[:, h, :], rhs=vaug_sb[:, kp, h, :],
                        start=(kp == 0 and h == 0),
                        stop=(last and h == H - 1))

            for kc in range(KC_N):
                x_sb = xpool.tile([P, NQC * H], BF, tag="x")  # [k, (q h)]
                # Per q-half (one PSUM bank each): S^T + bias, then exp.
                for g in range(G):
                    b_ps = b_psp.tile([P, TQ * H], FP, tag="b")  # [k,(q h)]
                    # S^T for all heads: two N=512 matmuls (hd halves)
                    for m in range(2):
                        nc.tensor.matmul(
                            b_ps,
                            lhsT=kT_sb[:, m, kc * P:(kc + 1) * P],
                            rhs=qblk_sb[:, m,
                                        g * TQ * H:(g + 1) * TQ * H],
                            start=(m == 0), stop=False)
                    # bias[k, (q, h)] += z[q]^T Wb, one z tile per query
                    z_sb = zpre[kc * G + g]
                    for t in range(TQ):
                        nc.tensor.matmul(b_ps[:, t * H:(t + 1) * H],
                                         lhsT=z_sb[:, t, :], rhs=wb_sb,
                                         start=False, stop=(t == TQ - 1))
                    # exp out of PSUM
                    nc.scalar.activation(
                        x_sb[:, g * TQ * H:(g + 1) * TQ * H], b_ps,
                        mybir.ActivationFunctionType.Exp)
                # attn.V lagged two chunks behind exp so the PE never
                # waits on ACT latency
                av_queue.append((x_sb, kc))
                if len(av_queue) > 2:
                    av_emit(av_queue.pop(0), False)
            while av_queue:
                av_emit(av_queue.pop(0), len(av_queue) == 0)

            # ---- epilogue: normalize, transpose, output projection ----
            recip_sb = const.tile([P, H], FP)
            for h in range(H):
                nc.vector.reciprocal(recip_sb[:, h:h + 1],
                                     o_ps[:, h * (D + 1) + D:h * (D + 1) + D + 1])
            o_sb = const.tile([P, 2, P], BF)     # [q, half, (h d)%128]
            for h in range(H):
                nc.vector.tensor_scalar(
                    o_sb[:, h // 4, (h % 4) * 32:(h % 4) * 32 + 32],
                    o_ps[:, h * (D + 1):h * (D + 1) + D],
                    recip_sb[:, h:h + 1], None, mult)
            oT_sb = const.tile([P, 2, P], BF)
            for m in range(2):
                t_full = proj_ps.tile([P, 512], BF, tag="proj", name="t_full")
                t_ps = t_full[:, :P]
                nc.tensor.transpose(t_ps, o_sb[:, m, :], ident_bf)
                nc.vector.tensor_copy(oT_sb[:, m, :], t_ps)
            ps = proj_ps.tile([P, 512], FP, tag="proj")
            for m in range(2):
                nc.tensor.matmul(ps[:, :CQ], lhsT=oT_sb[:, m, :],
                                 rhs=wp_sb[:, m, :], start=(m == 0), stop=False)
            nc.tensor.matmul(ps[:, :CQ], lhsT=ones_row, rhs=bp_sb,
                             start=False, stop=True)
            y_sb = const.tile([P, CQ], FP)
            nc.vector.tensor_copy(y_sb, ps[:, :CQ])
            nc.sync.dma_start(y[:], y_sb)

    nc.compile()
    return nc


def prep_inputs(x_q, x_kv, z, Wq, bq, Wkv, bkv, Wb, bb, Wp, bp):
    """Host-side shard prep.  Returns in_maps for the 8 cores.

    bb and the K half of bkv are constant along the softmax axis and
    cancel; they are not shipped.
    """
    xkvT = np.ascontiguousarray(x_kv[0].T).astype(NP_BF)     # [CKV, nk]
    shared = dict(xkvT=xkvT,
                  Wq=np.ascontiguousarray(Wq).astype(NP_BF),
                  bq=np.ascontiguousarray(bq, dtype=np.float32),
                  Wkv=np.ascontiguousarray(Wkv).astype(NP_BF),
                  bkvV=np.ascontiguousarray(bkv[H * D:]).astype(NP_BF),
                  Wb=np.ascontiguousarray(Wb).astype(NP_BF),
                  WpBF=np.ascontiguousarray(Wp).astype(NP_BF),
                  bpBF=np.ascontiguousarray(bp).astype(NP_BF))
    in_maps = []
    for i in range(NCORES):
        qs = i * NQC
        zi = z[0, qs:qs + NQC]                           # [q, k, c]
        # -> [kc, g, c, tq, k] with q = g*TQ + tq, key = kc*P + k
        zi = (zi.reshape(G, TQ, KC_N, P, BD)
                .transpose(2, 0, 4, 1, 3))
        in_maps.append(dict(
            zT=np.ascontiguousarray(zi).astype(NP_F8),
            xqT=np.ascontiguousarray(x_q[0, qs:qs + NQC].T).astype(NP_BF),
            **shared,
        ))
    return in_maps


_NC_CACHE = {}


def kernel(x_q, x_kv, z, Wq, bq, Wkv, bkv, Wb, bb, Wp, bp):
    key = "full"
    if key not in _NC_CACHE:
        _NC_CACHE[key] = build_program()
    nc = _NC_CACHE[key]
    in_maps = prep_inputs(x_q, x_kv, z, Wq, bq, Wkv, bkv, Wb, bb, Wp, bp)
    res = run_bass_kernel_spmd(nc, in_maps, list(range(NCORES)))
    out = np.empty((1, NQ, CQ), dtype=np.float32)
    for i in range(NCORES):
        out[0, i * NQC:(i + 1) * NQC, :] = res.results[i]["y"]
    return out


# revision 21
# speedup vs baseline: 1.2231x; 1.2231x over previous
"""BiasAttention TRN2 kernel — q-sharded across 8 NeuronCores, fp8 z.

Each core owns a block of 128 queries and computes the full attention for
them (all 8 heads, all 1024 keys).  The z-bias projection dominates both
HBM traffic and PE time, so z ships as fp8e3 (e3m4: 4 mantissa bits keeps
the bias quantization error ~5x below the bf16-path output error budget)
— halving DMA bytes vs bf16 and doubling LDWEIGHTS rate via fast weight
load.  Attention runs in transposed form S^T[k, q] so that P^T = exp(S^T)
is directly the stationary operand for attn.V — no PE transposes of P.
The z.Wb matmuls and the K^T.Q matmuls accumulate into one PSUM tile
[k, q, h]; ACT applies exp straight out of PSUM.  The bias-projection
bias bb and the K half of bkv are constant along the softmax axis and
cancel — they are dropped.
"""

import sys

if "/opt/trn_rl_repo" not in sys.path:
    sys.path.insert(0, "/opt/trn_rl_repo")

import ml_dtypes
import numpy as np

import concourse.bass as bass
import concourse.mybir as mybir
from concourse import bacc
from concourse.bass_utils import run_bass_kernel_spmd
from concourse.masks import make_identity
from concourse.tile import TileContext

P = 128          # partitions
H = 8            # heads
D = 32           # head dim
CQ = 256         # q channels
CKV = 256        # kv channels
BD = 128         # bias (z) channels
NQ = 1024        # total queries
NK = 1024        # total keys
NCORES = 8
NQC = NQ // NCORES   # queries per core = 128
KC_N = NK // P       # k-chunks = 8
G = 2                # z DMA groups per k-chunk
TQ = NQC // G        # q-tiles per group = 64
SCALE = D ** (-0.5)

FP = mybir.dt.float32
BF = mybir.dt.bfloat16
F8 = mybir.dt.float8e3
NP_BF = ml_dtypes.bfloat16
NP_F8 = ml_dtypes.float8_e3m4


def build_program():
    add = mybir.AluOpType.add
    mult = mybir.AluOpType.mult

    nc = bacc.Bacc("TRN2", target_bir_lowering=False, debug=False,
                   num_devices=NCORES)

    # ---- I/O ----
    # zT[kc, g, c, tq, k] = z[q = g*TQ + tq, kc*P + k, c]
    zT = nc.dram_tensor("zT", [KC_N, G, BD, TQ, P], F8, kind="ExternalInput")
    xqT = nc.dram_tensor("xqT", [CQ, NQC], BF, kind="ExternalInput")
    xkvT = nc.dram_tensor("xkvT", [CKV, NK], BF, kind="ExternalInput")
    Wq = nc.dram_tensor("Wq", [CQ, H * D], BF, kind="ExternalInput")
    bq = nc.dram_tensor("bq", [H * D], FP, kind="ExternalInput")
    Wkv = nc.dram_tensor("Wkv", [CKV, 2 * H * D], BF, kind="ExternalInput")
    bkvV = nc.dram_tensor("bkvV", [H * D], BF, kind="ExternalInput")
    Wb = nc.dram_tensor("Wb", [BD, H], BF, kind="ExternalInput")
    WpBF = nc.dram_tensor("WpBF", [H * D, CQ], BF, kind="ExternalInput")
    bpBF = nc.dram_tensor("bpBF", [CQ], BF, kind="ExternalInput")
    y = nc.dram_tensor("y", [NQC, CQ], FP, kind="ExternalOutput")

    with TileContext(nc) as tc:
        with (
            tc.tile_pool(name="const", bufs=1) as const,
            tc.tile_pool(name="zpool", bufs=KC_N * G) as zpool,
            tc.tile_pool(name="xpool", bufs=4) as xpool,
            tc.tile_pool(name="proj_ps", bufs=3, space="PSUM") as proj_ps,
            tc.tile_pool(name="b_ps", bufs=4, space="PSUM") as b_psp,
            tc.tile_pool(name="o_ps", bufs=1, space="PSUM") as o_psp,
        ):
            # ---- constants / weights to SBUF ----
            wb_sb = const.tile([P, H], BF)
            nc.sync.dma_start(wb_sb, Wb[:])
            wq_sb = const.tile([P, 2, H * D], BF)
            nc.sync.dma_start(wq_sb, Wq.rearrange("(o p) m -> p o m", p=P))
            wkv_sb = const.tile([P, 2, 2 * H * D], BF)
            nc.scalar.dma_start(wkv_sb, Wkv.rearrange("(o p) m -> p o m", p=P))
            wp_sb = const.tile([P, 2, CQ], BF)
            nc.scalar.dma_start(wp_sb, WpBF.rearrange("(o p) m -> p o m", p=P))
            xqT_sb = const.tile([P, 2, NQC], BF)
            nc.sync.dma_start(xqT_sb, xqT.rearrange("(o p) q -> p o q", p=P))
            xkvT_sb = const.tile([P, 2, NK], BF)
            nc.scalar.dma_start(xkvT_sb, xkvT.rearrange("(o p) k -> p o k", p=P))
            bq_sb = const.tile([P, 2], FP)
            nc.sync.dma_start(bq_sb, bq.rearrange("(o p) -> p o", p=P))
            bkvV_sb = const.tile([1, H * D], BF)
            nc.sync.dma_start(bkvV_sb, bkvV[None, :])
            bp_sb = const.tile([1, CQ], BF)
            nc.sync.dma_start(bp_sb, bpBF[None, :])
            # all 16 z groups resident: every DMA issued up front so the
            # DMA engines stream back-to-back at line rate.  The first two
            # groups are split into 256KB quarters so the bias matmuls can
            # begin as soon as the first piece lands.
            # z streams on both HWDGE rings (Sync + Scalar).  Each ring has
            # 4 semaphore lanes with one outstanding DMA each, and every
            # dma_start *instruction* inline-waits its lane — so only the
            # first few groups are issued up front; the rest trickle from
            # inside the main loop, keeping <=3 in flight per ring and the
            # issuing engines (incl. ACT, which also runs the exps) free.
            ZPRE = 6
            zpre = [zpool.tile([P, TQ, P], F8, tag="z", name=f"z{g}")
                    for g in range(KC_N * G)]

            def z_issue(gidx):
                eng = nc.sync if gidx % 2 == 0 else nc.scalar
                z_sb = zpre[gidx]
                if gidx < 2:
                    sq = TQ // 4
                    for s in range(4):
                        eng.dma_start(
                            z_sb[:, s * sq:(s + 1) * sq, :],
                            zT[gidx // G, gidx % G, :, s * sq:(s + 1) * sq, :])
                else:
                    eng.dma_start(z_sb, zT[gidx // G, gidx % G])

            for gidx in range(ZPRE):
                z_issue(gidx)
            ident_bf = const.tile([P, P], BF)
            make_identity(nc, ident_bf)
            ones_row = const.tile([1, P], BF)
            nc.vector.memset(ones_row, 1.0)

            # V augmented with a ones column per head: [k, kc, h, D+1]
            vaug_sb = const.tile([P, KC_N, H, D + 1], BF)
            nc.vector.memset(vaug_sb, 1.0)

            # ---- projections (bf16 in, fp32 psum accumulate) ----
            # Q block-diag: qblk[(hl d), m, q*H + h] = Q_scaled[q, h, d] when
            # h == m*4 + hl, else 0.  Makes S^T for all 8 heads two plain
            # N=512 matmuls per (k-chunk, q-half) with output [k, (q, h)]
            # contiguous — the same layout the bias matmuls accumulate into.
            qblk_sb = const.tile([P, 2, NQC * H], BF)
            nc.vector.memset(qblk_sb, 0.0)
            qblk_v = qblk_sb.rearrange("p m (q h) -> p m q h", h=H)
            for m in range(2):
                ps = proj_ps.tile([P, 512], FP, tag="proj")
                for c in range(2):
                    nc.tensor.matmul(ps[:, :NQC],
                                     lhsT=wq_sb[:, c, m * P:(m + 1) * P],
                                     rhs=xqT_sb[:, c, :],
                                     start=(c == 0), stop=(c == 1))
                for hl in range(4):
                    r0 = hl * 32
                    nc.vector.tensor_scalar(
                        qblk_v[r0:r0 + 32, m, :, m * 4 + hl],
                        ps[r0:r0 + 32, :NQC],
                        bq_sb[r0:r0 + 32, m:m + 1], SCALE, add, mult)

            # K^T [(h d), k], no bias (cancels in softmax), stored bf16
            kT_sb = const.tile([P, 2, NK], BF)
            for m in range(2):
                for nh in range(NK // 512):
                    ps = proj_ps.tile([P, 512], FP, tag="proj")
                    for c in range(2):
                        nc.tensor.matmul(ps[:, :],
                                         lhsT=wkv_sb[:, c, m * P:(m + 1) * P],
                                         rhs=xkvT_sb[:, c, nh * 512:(nh + 1) * 512],
                                         start=(c == 0), stop=(c == 1))
                    nc.vector.tensor_copy(
                        kT_sb[:, m, nh * 512:(nh + 1) * 512], ps)

            # V [k, (h d)] + bkv_V, written into vaug (ones col preserved)
            for kc in range(KC_N):
                ps = proj_ps.tile([P, 512], FP, tag="proj", name="v_ps")
                for c in range(2):
                    nc.tensor.matmul(ps[:, :H * D],
                                     lhsT=xkvT_sb[:, c, kc * P:(kc + 1) * P],
                                     rhs=wkv_sb[:, c, H * D:2 * H * D],
                                     start=(c == 0), stop=False)
                nc.tensor.matmul(ps[:, :H * D], lhsT=ones_row,
                                 rhs=bkvV_sb, start=False, stop=True)
                nc.vector.tensor_copy(
                    vaug_sb[:, kc, :, 0:D],
                    ps[:, :H * D].rearrange("p (h d) -> p h d", h=H))

            # ---- main loop over k-chunks: S^T + bias accumulate in PSUM ----
            o_ps = o_psp.tile([P, H * (D + 1)], FP)   # [q, h*(D+1)]
            av_queue = []

            def av_emit(item, last):
                xp, kp = item
                xv = xp.rearrange("p (q h) -> p h q", h=H)
                for h in range(H):
                    nc.tensor.matmul(
                        o_ps[:, h * (D + 1):(h + 1) * (D + 1)],
                        lhsT=xv[:, h, :], rhs=vaug_sb[:, kp, h, :],
                        start=(kp == 0 and h == 0),
                        stop=(last and h == H - 1))

            for kc in range(KC_N):
                x_sb = xpool.tile([P, NQC * H], BF, tag="x")  # [k, (q h)]
                # Per q-half (one PSUM bank each): S^T + bias, then exp.
                for g in range(G):
                    b_ps = b_psp.tile([P, TQ * H], FP, tag="b")  # [k,(q h)]
                    # S^T for all heads: two N=512 matmuls (hd halves)
                    for m in range(2):
                        nc.tensor.matmul(
                            b_ps,
                            lhsT=kT_sb[:, m, kc * P:(kc + 1) * P],
                            rhs=qblk_sb[:, m,
                                        g * TQ * H:(g + 1) * TQ * H],
                            start=(m == 0), stop=False)
                    # bias[k, (q, h)] += z[q]^T Wb, one z tile per query
                    z_sb = zpre[kc * G + g]
                    for t in range(TQ):
                        nc.tensor.matmul(b_ps[:, t * H:(t + 1) * H],
                                         lhsT=z_sb[:, t, :], rhs=wb_sb,
                                         start=False, stop=(t == TQ - 1))
                    # exp out of PSUM
                    nc.scalar.activation(
                        x_sb[:, g * TQ * H:(g + 1) * TQ * H], b_ps,
                        mybir.ActivationFunctionType.Exp)
                    # trickle-issue the z DMA a few groups ahead
                    nxt = kc * G + g + ZPRE
                    if nxt < KC_N * G:
                        z_issue(nxt)
                # attn.V lagged two chunks behind exp so the PE never
                # waits on ACT latency
                av_queue.append((x_sb, kc))
                if len(av_queue) > 2:
                    av_emit(av_queue.pop(0), False)
            while av_queue:
                av_emit(av_queue.pop(0), len(av_queue) == 0)

            # ---- epilogue: normalize, transpose, output projection ----
            recip_sb = const.tile([P, H], FP)
            for h in range(H):
                nc.vector.reciprocal(recip_sb[:, h:h + 1],
                                     o_ps[:, h * (D + 1) + D:h * (D + 1) + D + 1])
            o_sb = const.tile([P, 2, P], BF)     # [q, half, (h d)%128]
            for h in range(H):
                nc.vector.tensor_scalar(
                    o_sb[:, h // 4, (h % 4) * 32:(h % 4) * 32 + 32],
                    o_ps[:, h * (D + 1):h * (D + 1) + D],
                    recip_sb[:, h:h + 1], None, mult)
            oT_sb = const.tile([P, 2, P], BF)
            for m in range(2):
                t_full = proj_ps.tile([P, 512], BF, tag="proj", name="t_full")
                t_ps = t_full[:, :P]
                nc.tensor.transpose(t_ps, o_sb[:, m, :], ident_bf)
                nc.vector.tensor_copy(oT_sb[:, m, :], t_ps)
            ps = proj_ps.tile([P, 512], FP, tag="proj")
            for m in range(2):
                nc.tensor.matmul(ps[:, :CQ], lhsT=oT_sb[:, m, :],
                                 rhs=wp_sb[:, m, :], start=(m == 0), stop=False)
            nc.tensor.matmul(ps[:, :CQ], lhsT=ones_row, rhs=bp_sb,
                             start=False, stop=True)
            y_sb = const.tile([P, CQ], FP)
            nc.vector.tensor_copy(y_sb, ps[:, :CQ])
            nc.sync.dma_start(y[:], y_sb)

    nc.compile()
    return nc


def prep_inputs(x_q, x_kv, z, Wq, bq, Wkv, bkv, Wb, bb, Wp, bp):
    """Host-side shard prep.  Returns in_maps for the 8 cores.

    bb and the K half of bkv are constant along the softmax axis and
    cancel; they are not shipped.
    """
    xkvT = np.ascontiguousarray(x_kv[0].T).astype(NP_BF)     # [CKV, nk]
    shared = dict(xkvT=xkvT,
                  Wq=np.ascontiguousarray(Wq).astype(NP_BF),
                  bq=np.ascontiguousarray(bq, dtype=np.float32),
                  Wkv=np.ascontiguousarray(Wkv).astype(NP_BF),
                  bkvV=np.ascontiguousarray(bkv[H * D:]).astype(NP_BF),
                  Wb=np.ascontiguousarray(Wb).astype(NP_BF),
                  WpBF=np.ascontiguousarray(Wp).astype(NP_BF),
                  bpBF=np.ascontiguousarray(bp).astype(NP_BF))
    in_maps = []
    for i in range(NCORES):
        qs = i * NQC
        zi = z[0, qs:qs + NQC]                           # [q, k, c]
        # -> [kc, g, c, tq, k] with q = g*TQ + tq, key = kc*P + k
        zi = (zi.reshape(G, TQ, KC_N, P, BD)
                .transpose(2, 0, 4, 1, 3))
        in_maps.append(dict(
            zT=np.ascontiguousarray(zi).astype(NP_F8),
            xqT=np.ascontiguousarray(x_q[0, qs:qs + NQC].T).astype(NP_BF),
            **shared,
        ))
    return in_maps


_NC_CACHE = {}


def kernel(x_q, x_kv, z, Wq, bq, Wkv, bkv, Wb, bb, Wp, bp):
    key = "full"
    if key not in _NC_CACHE:
        _NC_CACHE[key] = build_program()
    nc = _NC_CACHE[key]
    in_maps = prep_inputs(x_q, x_kv, z, Wq, bq, Wkv, bkv, Wb, bb, Wp, bp)
    res = run_bass_kernel_spmd(nc, in_maps, list(range(NCORES)))
    out = np.empty((1, NQ, CQ), dtype=np.float32)
    for i in range(NCORES):
        out[0, i * NQC:(i + 1) * NQC, :] = res.results[i]["y"]
    return out
